# revision 1
# baseline (speedup 1.0000x reference)
"""Trainium2 Bass kernel for nn_CelltypeScaleLayer (segment gather + scale + transpose).

Reference computation:
    z = x[idx.reshape(-1)] * repeat(weight, M)[:, None]   # (NJ, NCELL)
    out = z.T.reshape(-1)                                 # (NCELL * NJ,)

Sharding: data-parallel over the NJ (gathered-row) axis. Core m owns output
columns j in [m*JPC, (m+1)*JPC) of the (NCELL, NJ) output, i.e. a contiguous
slab of the flattened output. x is replicated; idx/weight shards are tiny and
pre-laid-out on the host (int16 wrapped index layout for SWDGE dma_gather, and
a per-128-j-group weight table).

Per-core pipeline, per chunk of CHUNK j positions:
  1. SWDGE dma_gather: rows x[idx[j], :] (2KB each) from HBM into SBUF,
     slot layout gb[j%128, j//128, :].
  2. Per 128-j group: scalar-engine multiply by the per-j weight
     (per-partition scalar from the host-prepared table).
  3. Per 128-column block q: PE transpose (128j, 128c) -> PSUM (128c, 128j).
  4. DVE copy PSUM -> output staging buffer (partition = cell column).
  5. HWDGE DMA staging -> HBM output rows (contiguous along j).

Measured (8 cores, axon trn2): ~142-153 us per full problem (repeat-delta
wall-clock; TimelineSim cost model predicts 158 us) — at the ~143 us two-pass
HBM roofline (25.6MB gather read + 25.6MB write per core @ ~358 GB/s per NC).
Shared-device contention on the axon terminal makes repeated measurements
fluctuate (best observed ~96 us, worst ~190 us per iteration).
"""

import numpy as np

import concourse.bacc as bacc
import concourse.tile as tile
import concourse.mybir as mybir
from concourse import masks
from concourse.bass_utils import run_bass_kernel_spmd

F32 = mybir.dt.float32
I16 = mybir.dt.int16

# Problem shape (hardcoded per the harness contract).
NF = 20000        # x rows (features)
NCELL = 512       # x cols (cells) == output rows
NCT = 50          # celltypes
M = 2000          # rows gathered per celltype
NJ = NCT * M      # 100000 gathered rows == output cols

NCORES = 8
JPC = NJ // NCORES          # 12500 output columns per core
CHUNK = 896                 # gather indices per dma_gather call (7 groups of 128).
                            # >=1280 per SWDGE gather crashes the device
                            # (descriptor-ring capacity is 1024 entries).
GPC = CHUNK // 128          # groups per chunk
NCHUNK = -(-JPC // CHUNK)   # 14
NIDX = NCHUNK * CHUNK       # 12544 (tail padded with -1, skipped by the DMA)
NQ = NCELL // 128           # 4 column blocks

_cached = None


def _build(repeats=1, ncores=NCORES):
    """Build + compile the SPMD program. `repeats` re-runs the whole pipeline
    that many times inside one NEFF (used only for timing measurements)."""
    nc = bacc.Bacc("TRN2", target_bir_lowering=False, debug=False,
                   num_devices=ncores)
    x = nc.dram_tensor("x", [NF, NCELL], F32, kind="ExternalInput")
    idxs = nc.dram_tensor("idxs", [128, NIDX // 16], I16, kind="ExternalInput")
    wtbl = nc.dram_tensor("wtbl", [128, NIDX // 128], F32, kind="ExternalInput")
    out = nc.dram_tensor("out", [NCELL, JPC], F32, kind="ExternalOutput")

    with tile.TileContext(nc) as tc:
        with tc.tile_pool(name="const", bufs=1) as cpool:
            ident = cpool.tile([128, 128], F32)
            masks.make_identity(nc, ident[:])
            idx_sb = cpool.tile([128, NIDX // 16], I16)
            nc.sync.dma_start(idx_sb[:], idxs.ap())
            wtbl_sb = cpool.tile([128, NIDX // 128], F32)
            nc.sync.dma_start(wtbl_sb[:], wtbl.ap())

            with (
                tc.tile_pool(name="gpool", bufs=4) as gpool,
                tc.tile_pool(name="opool", bufs=3) as opool,
                tc.tile_pool(name="pspool", bufs=8, space="PSUM") as pspool,
            ):
                for _ in range(repeats):
                    for k in range(NCHUNK):
                        nvalid = min(JPC - k * CHUNK, CHUNK)
                        gb = gpool.tile([128, GPC, NCELL], F32, tag="gb")
                        if nvalid < CHUNK:
                            # zero the last group so padded tail slots hold no
                            # garbage (those columns are never DMA'd out, but
                            # they do flow through mul/transpose)
                            nc.vector.memset(gb[:, GPC - 1, :], 0.0)
                        nc.gpsimd.dma_gather(
                            gb[:],
                            x.ap(),
                            idx_sb[:, k * (CHUNK // 16):(k + 1) * (CHUNK // 16)],
                            CHUNK,
                            nvalid,
                            NCELL,
                        )
                        ob = opool.tile([128, NQ, CHUNK], F32, tag="ob")
                        for g in range(GPC):
                            gcol = k * GPC + g
                            nc.scalar.activation(
                                gb[:, g, :], gb[:, g, :],
                                mybir.ActivationFunctionType.Copy,
                                scale=wtbl_sb[:, gcol:gcol + 1],
                            )
                            ps = pspool.tile([128, 512], F32, tag="ps")
                            for q in range(NQ):
                                nc.tensor.transpose(
                                    ps[:, q * 128:(q + 1) * 128],
                                    gb[:, g, q * 128:(q + 1) * 128],
                                    ident[:],
                                )
                            ps_v = ps[:, :].rearrange("p (q j) -> p q j", q=NQ)
                            nc.vector.tensor_copy(
                                ob[:, :, g * 128:(g + 1) * 128], ps_v)
                        for q in range(NQ):
                            nc.sync.dma_start(
                                out.ap()[q * 128:(q + 1) * 128,
                                         k * CHUNK:k * CHUNK + nvalid],
                                ob[:, q, :nvalid],
                            )
    nc.compile()
    return nc


def _host_prep(x, weight, idx, ncores=NCORES):
    x = np.ascontiguousarray(np.asarray(x), dtype=np.float32)
    weight = np.asarray(weight, dtype=np.float32)
    idx_flat = np.asarray(idx).reshape(-1).astype(np.int64)
    w_exp = np.repeat(weight, M).astype(np.float32)  # (NJ,) per-j weight

    in_maps = []
    for m in range(ncores):
        j0 = m * JPC
        padded = np.full((NIDX,), -1, dtype=np.int64)
        padded[:JPC] = idx_flat[j0:j0 + JPC]
        # dma_gather index layout: index i lives at partition i%16, free i//16,
        # replicated across the 8 Q7 core groups.
        wrapped16 = padded.reshape(NIDX // 16, 16).T.astype(np.int16)
        wrapped = np.ascontiguousarray(np.tile(wrapped16, (8, 1)))  # (128, NIDX//16)

        # per-128-j-group weight table: wtbl[p, grp] = w for j = grp*128 + p
        wpad = np.ones((NIDX,), dtype=np.float32)
        wpad[:JPC] = w_exp[j0:j0 + JPC]
        wtbl = np.ascontiguousarray(wpad.reshape(NIDX // 128, 128).T)

        in_maps.append({"x": x, "idxs": wrapped, "wtbl": wtbl})
    return in_maps


def _run(inputs):
    global _cached
    if _cached is None:
        _cached = _build()
    nc = _cached
    in_maps = _host_prep(inputs["x"], inputs["weight"], inputs["idx"])
    res = run_bass_kernel_spmd(nc, in_maps, list(range(NCORES)))
    parts = [res.results[m]["out"] for m in range(NCORES)]
    full = np.concatenate(parts, axis=1)  # (NCELL, NJ)
    return np.ascontiguousarray(full).reshape(-1), res


def kernel(**inputs) -> np.ndarray:
    out, _ = _run(inputs)
    return out

